# revision 42
# baseline (speedup 1.0000x reference)
"""Distributed single-head causal attention for TRN2 (8 NeuronCores).

Problem: x[B=4, T=4096, C=768], Wq/Wk/Wv[H=64, C] ->
  out[b,t,:] = softmax(causal(q k^T * C^-0.05)) @ v   (single head)

Sharding: core ci = (batch b = ci//2, interleave half h = ci%2). Each core
computes k/v for its whole batch and attention for the 16 q-tiles {2m+h}.

All 8 cores run ONE graph (uniform SPMD); every per-core difference is
carried in per-core DRAM inputs (a per-core COLUMN PERMUTATION of x and the
0/1 P-mask), never in instruction-stream structure or AP offsets.

v3: S matmuls are 2x ROW-PACKED (tile_position).  The S contraction is
only H=64 wide, so two S matmuls (pair elements a, b) run CONCURRENTLY
on disjoint 64-row halves of the PE array: a uses array rows 0:63
(tile_position (0,0)), b uses rows 64:127 ((64,0)).  Layout to feed
them with zero extra copies:
  - wq = [Wq | Wq]: qt rows 64:128 carry a second q copy (was zeros).
  - kv proj weights differ by half: half 0 = [Wk|Wv] (k in rows 0:64),
    half 1 = [Wv|Wk] (k in rows 64:128).  Every pair is (first-half
    chunk, second-half chunk): diag pairs (d, d+4) already were; full
    pairs re-paired (2p,2p+1) -> (8m+p, 8m+p+4).  vtrans picks the v
    rows per half (64:128 for h0, 0:64 for h1).  Masks unchanged.
Effect: an S pair costs ~max(w)/2.4GHz instead of 2w/2.4GHz; measured
PE time drops ~7us and the exp (ACT) stream starts earlier.

v4-v6 (head + stream, from NTFF traces: the kernel is exp/ACT-paced in
steady state -- 40 ACTIVATEs ~= 40us busy at 1 elem/cycle/lane @1.2GHz
-- and the head was DMA-rate-bound):
  - ONE compact [C,192]=[Wq|Wk|Wv] weight DMA (294KB, not 3x196KB);
    the [Wq|Wq]/[Wk|Wv]/[Wv|Wk] matmul layouts are assembled on-chip
    by 6 strided DVE copies.
  - li=0's x arrives as twelve 512-col HALVES spread over THREE DMA
    queues (sync/gpsimd/scalar -- vector can't initiate DMAs; two
    queues only reach ~240GB/s in the ramp window, three ~330).  x(1)
    also rides all three queues so qproj(1) isn't DMA-gated.
  - block-0 head: q, kv-h0, kv-h1 accumulate CONCURRENTLY in 3 psum
    banks, consuming each chunk-half the moment it lands; the three
    PSUM->SBUF drains run on DVE/DVE/scalar in parallel; vtrans rides
    att(0)'s first two windows.  First real exp ~17us (was 24.7).
  - diag S plane 1 is written COMPACTED at [0:w] so every pair's exp
    is one contiguous AP (strided 3D costs ACT ~100ns/instr).
  - norm path in bf16 (po holds unnormalized exp sums up to ~e^51, so
    f16 would overflow): osb/pot/id32 bf16, ~2us less PE.
  - kv of blocks 2-3 weaves as 3-MM substeps over windows 1-6 (block 1
    stays coarse: its kv must finish by window 5).
Scheduling notes from failed experiments (v7-v10, all REGRESSED --
this emission order is load-bearing): deferring block 0's O-pairs,
double-popping pending drains in diag windows, splitting qproj into
2-MM slices, and moving vtrans after both kv copies each made things
1.7-4.5us WORSE; ptile bufs=8 also regressed vs 6.  PSUM is the hard
constraint (8 banks: pss 2x2 + pskv + psv + pso 2) that blocks wider
exp batching; DVE/gpsimd cannot do exp, so ACT ~40us busy is a floor.
Measured v6: 75815ns (v2 baseline 81881ns), rel err 3.26e-3 (bf16
norm; gate 2e-2).  Remaining: ~10us of exp-stream gaps at block
transitions (PE-queue congestion), ~2.8us drain tail, ~4.9us barrier
postamble, ~8us preamble-to-first-DMA.

v2 design (from the v1 trace: PE idle at start, HAM half-clock windows,
exp-paced attention, DMA issue overhead, serialized tail):
  - x is streamed ONCE as 24 [128,1024] f16 chunks (no separate xq stream).
    Host permutes columns per-core so block li = [my 4 q-tiles | partner 4
    tiles]; the q projection reads the fixed [:, 0:512] slice of the same
    chunks the kv projection consumes.  DMA drops 9.2MB -> 6.6MB and all
    chunk DMAs are emitted up front on both queues.
  - exact-causal trim: within the diagonal block, chunk d (0..7) only
    multiplies q-tiles >= tl_min(d); S matmul, exp and O matmul all shrink
    together.  Diagonal chunks pair (d, d+4) -> equal widths -> one strided
    exp per pair ([128,2,w] AP).
  - causal masking = one 128-wide 0/1 bf16 multiply on P per diagonal chunk
    (gpsimd/Pool engine), replacing 256-wide f32 PSUM adds on DVE.
  - warmup burst shrunk 16->6 matmuls (exp-LUT preload kept).
  - tail: O^T transposes land in disjoint slices of one PSUM tile (no
    serialize), one batched output DMA per li ([512,64] each).
  - lazy drains: each li's exp-gated tail O-pairs + normalize are emitted
    between the NEXT li's S-pairs, so the in-order PE never stalls on them.
  - each block's kv groups ride its own attention window (pairs 1-4), not
    the previous one's (whose chunks may not have landed).
Precision: f16 q/k/x/W, bf16 P/V, f32 elsewhere.  No row-max subtraction
(masked scores stay in [-53,51]; exp exact in f32).

Measured on trn2 (neuron-profile, whole NEFF): ~82-83us per core (v1
baseline 92.8-93.8us), rel err 2.28e-3 (gate 2e-2).  Fixed framework
overhead inside the measured window: ~6.8us preamble-to-first-DMA +
~7.5us postamble (8-way engine barrier + ~51 semaphore resets).
Aggregate DMA is ~210GB/s shared across all queues (one AXI port), so
the first ~15us are DMA-bound: scratch 'filler' matmuls pad the PE there
to keep the HAM clock governor at full speed (it demotes the core to
half clock after ~2-3us of PE idleness and needs ~4-5us of sustained
activity to promote).
"""

import sys

for _p in ("/opt/trn_rl_repo",):
    if _p not in sys.path:
        sys.path.insert(0, _p)

import ml_dtypes
import numpy as np

import concourse.bass as bass  # noqa: F401  (registers engine classes)
import concourse.tile as tile
from concourse import bacc, mybir
from concourse.bass_utils import run_bass_kernel_spmd

B, T, C, H = 4, 4096, 768, 64
NCORES = 8
SCALE = float(C ** (-0.05))
CCH = C // 128          # 6 contraction chunks
NSC = T // 128          # 32 s-chunks
TQ = T // 2             # 2048 q columns per core
NWARM = 8               # warmup matmuls (PE clock ramp, ~3.4us cold)

F32 = mybir.dt.float32
BF16 = mybir.dt.bfloat16
F16 = mybir.dt.float16
EXP = mybir.ActivationFunctionType.Exp

_CACHE: dict = {}

# diagonal-chunk trim: chunk d of a block only hits q-tiles >= TLMIN[d]
TLMIN = [0, 1, 2, 3, 0, 1, 2, 3]


def _install_ntff_hook():
    """Provide antenv.axon_hooks if the image lacks it, so
    run_bass_kernel_spmd(trace=True) can capture NTFF profiles under axon."""
    try:
        from antenv.axon_hooks import get_axon_ntff_profile_hook  # noqa: F401
        return  # already present
    except ImportError:
        pass
    import contextlib
    import ctypes
    import types

    so_path = "/opt/axon/libaxon_pjrt.so"
    mod = types.ModuleType("antenv.axon_hooks")
    _state = {"hook": None}
    mod.set_axon_ntff_profile_hook = lambda h: _state.__setitem__("hook", h)
    mod.get_axon_ntff_profile_hook = lambda: _state["hook"]
    try:
        lib = ctypes.CDLL(so_path)
        if hasattr(lib, "axon_start_nrt_profile"):
            lib.axon_start_nrt_profile.argtypes = [
                ctypes.POINTER(ctypes.c_int64), ctypes.c_size_t]
            lib.axon_start_nrt_profile.restype = ctypes.c_int64
            lib.axon_stop_nrt_profile.argtypes = [ctypes.c_char_p]
            lib.axon_stop_nrt_profile.restype = ctypes.c_int64

            @contextlib.contextmanager
            def _hook(output_dir, device_ids):
                import jax
                jax.devices()
                if device_ids:
                    ids = (ctypes.c_int64 * len(device_ids))(*device_ids)
                    rc = lib.axon_start_nrt_profile(ids, len(device_ids))
                else:
                    rc = lib.axon_start_nrt_profile(None, 0)
                if rc != 0:
                    raise RuntimeError(f"axon_start_nrt_profile rc={rc}")
                try:
                    yield
                finally:
                    n = lib.axon_stop_nrt_profile(str(output_dir).encode())
                    print(f"profile: {n} file(s) written to {output_dir}")

            _state["hook"] = _hook
    except OSError:
        pass
    import antenv
    sys.modules["antenv.axon_hooks"] = mod
    antenv.axon_hooks = mod


_install_ntff_hook()


def _build_graph():
    nc = bacc.Bacc("TRN2", target_bir_lowering=False, debug=False,
                   num_devices=NCORES)

    xt_d = nc.dram_tensor("xt", [C, T], F16, kind="ExternalInput")
    wraw_d = nc.dram_tensor("wraw", [C, 192], F16, kind="ExternalInput")
    m01_d = nc.dram_tensor("m01", [128, 2 * 128], BF16, kind="ExternalInput")
    id_d = nc.dram_tensor("ident", [128, 128], F16, kind="ExternalInput")
    id32_d = nc.dram_tensor("ident32", [H + 1, H + 1], BF16, kind="ExternalInput")
    out_d = nc.dram_tensor("out", [TQ, H], F32, kind="ExternalOutput")

    with tile.TileContext(nc) as tc:
        with (
            tc.tile_pool(name="consts", bufs=1) as consts,
            tc.tile_pool(name="persist", bufs=1) as persist,
            tc.tile_pool(name="ptile", bufs=6) as ptile,
            tc.tile_pool(name="opost", bufs=4) as opost,
            tc.tile_pool(name="pskv", bufs=1, space="PSUM") as pskv,
            tc.tile_pool(name="psv", bufs=1, space="PSUM") as psv,
            tc.tile_pool(name="pss", bufs=2, space="PSUM") as pss,
            tc.tile_pool(name="pso", bufs=2, space="PSUM") as pso,
        ):
            # ---- constants + x stream, in strict consumption order across
            # both DMA queues (aggregate DMA bw is ~210GB/s shared; the
            # startup is DMA-bound so queue order = need order).
            wq_t = consts.tile([128, CCH * 128], F16, tag="wq", name="wq_t")
            wkv_t = consts.tile([128, CCH * 128], F16, tag="wkv", name="wkv_t")
            wvk_t = consts.tile([128, CCH * 128], F16, tag="wvk", name="wvk_t")
            wraw_t = consts.tile([128, CCH * 192], F16, tag="wraw",
                                 name="wraw_t")
            id_t = consts.tile([128, 128], F16, tag="ident", name="id_t")
            id32_t = consts.tile([H + 1, H + 1], BF16, tag="id32",
                                 name="id32_t")
            m01_t = consts.tile([128, 2 * 128], BF16, tag="m01", name="m01_t")

            # head DMA: the first exp needs weights + ALL of x(li=0), so the
            # head is DMA-rate-bound.  Ship ONE compact [C,192]=[Wq|Wk|Wv]
            # weight tensor (294KB, the [Wq|Wq]/[Wk|Wv]/[Wv|Wk] layouts are
            # assembled on-chip by DVE), and spread li=0's twelve 512-col
            # chunk-halves over FOUR DMA queues (sync/vector/scalar/gpsimd)
            # -- two queues only reach ~240GB/s in the ramp-up window.
            nc.sync.dma_start(
                wraw_t[:].rearrange("p (c m) -> p c m", c=CCH),
                wraw_d.ap().rearrange("(c p) m -> p c m", p=128))
            xs = [[None] * CCH for _ in range(4)]
            x0h = [[None] * 2 for _ in range(CCH)]

            def dma_chunk(li, c, q):
                t_ = persist.tile([128, 1024], F16, tag=f"xs{li}_{c}",
                                  name=f"xs{li}_{c}")
                q.dma_start(t_[:], xt_d.ap()[c * 128:(c + 1) * 128,
                                             li * 1024:(li + 1) * 1024])
                xs[li][c] = t_

            def dma_half(c, half, q):
                t_ = persist.tile([128, 512], F16, tag=f"x0h{c}_{half}",
                                  name=f"x0h{c}_{half}")
                q.dma_start(t_[:], xt_d.ap()[c * 128:(c + 1) * 128,
                                             half * 512:(half + 1) * 512])
                x0h[c][half] = t_

            # need-order: h0 halves feed q+kv0, h1 halves feed kv1.  Only
            # sync/gpsimd/scalar can initiate DMAs.  x(1) also rides all
            # THREE queues: att(0)'s exps end ~21.5us and qproj(1) gates
            # att(1)'s first exp, so x(1) must be fully landed by ~19us.
            # The scalar queue's issue instructions all run before its
            # first real work (table load + exp stream).
            dma_half(0, 0, nc.scalar)
            dma_half(1, 0, nc.gpsimd)
            dma_half(2, 0, nc.sync)
            dma_half(3, 0, nc.scalar)
            dma_half(4, 0, nc.gpsimd)
            dma_half(5, 0, nc.sync)
            dma_half(0, 1, nc.scalar)
            dma_half(1, 1, nc.gpsimd)
            dma_half(2, 1, nc.gpsimd)
            dma_half(3, 1, nc.sync)
            dma_half(4, 1, nc.scalar)
            dma_half(5, 1, nc.gpsimd)
            nc.gpsimd.dma_start(id_t[:], id_d.ap()[:, :])
            nc.gpsimd.dma_start(m01_t[:], m01_d.ap()[:, :])
            dma_chunk(1, 2, nc.sync)
            dma_chunk(1, 1, nc.scalar)
            dma_chunk(1, 0, nc.gpsimd)
            dma_chunk(1, 5, nc.sync)
            dma_chunk(1, 4, nc.scalar)
            dma_chunk(1, 3, nc.gpsimd)
            nc.gpsimd.dma_start(id32_t[:], id32_d.ap()[:, :])
            for li in range(2, 4):
                for c in (0, 2, 4):
                    dma_chunk(li, c, nc.sync)
            for li in range(2, 4):
                for c in (1, 3, 5):
                    dma_chunk(li, c, nc.gpsimd)

            # assemble the 3 weight layouts from the compact DMA (DVE,
            # strided over the 6 contraction chunks; q is duplicated)
            wr3 = wraw_t[:].rearrange("p (c m) -> p c m", c=CCH)
            wq3 = wq_t[:].rearrange("p (c m) -> p c m", c=CCH)
            wkv3 = wkv_t[:].rearrange("p (c m) -> p c m", c=CCH)
            wvk3 = wvk_t[:].rearrange("p (c m) -> p c m", c=CCH)
            nc.vector.tensor_copy(wq3[:, :, 0:64], wr3[:, :, 0:64])
            nc.vector.tensor_copy(wq3[:, :, 64:128], wr3[:, :, 0:64])
            nc.vector.tensor_copy(wkv3[:, :, 0:64], wr3[:, :, 64:128])
            nc.vector.tensor_copy(wkv3[:, :, 64:128], wr3[:, :, 128:192])
            nc.vector.tensor_copy(wvk3[:, :, 0:64], wr3[:, :, 128:192])
            nc.vector.tensor_copy(wvk3[:, :, 64:128], wr3[:, :, 64:128])

            # ---- warmup: preload Exp LUT + wake the PE clock while DMAs
            # stream (writes scratch nothing reads).  fill() emits keep-alive
            # matmuls: the HAM clock governor demotes the core to half speed
            # after ~2us of PE idleness and needs ~5us of sustained activity
            # to promote, so the DMA-bound start is padded with scratch work.
            wsc = persist.tile([128, 512], F16, tag="wsc", name="wsc")
            nc.vector.memset(wsc[:], 0.25)
            wact = persist.tile([128, 64], F32, tag="wact", name="wact")
            nc.vector.memset(wact[:], 0.5)
            nc.scalar.activation(wact[:], wact[:], EXP, scale=SCALE)

            def fill(n):
                for _ in range(n):
                    wps = pss.tile([128, 2, 512], F32, tag="s", name="wps")
                    nc.tensor.matmul(wps[:, 0, :], lhsT=wsc[:, 0:128],
                                     rhs=wsc[:], start=True, stop=True)

            fill(NWARM)

            # ---- persistent intermediates ----
            kvt = persist.tile([128, T], F16, tag="kvt", name="kvt")
            qt = persist.tile([128, TQ], F16, tag="qt", name="qt")
            vaug = persist.tile([128, NSC * (H + 1)], BF16, tag="vaug",
                               name="vaug")
            # ones column of V_aug (accumulates the softmax denominator):
            # single strided memset over all 32 chunks
            nc.vector.memset(vaug.rearrange(
                "p (sc w) -> p sc w", w=H + 1)[:, :, H:H + 1], 1.0)

            # ---- projection phase bodies (per 1024-col block li) ----
            def emit_proj_q(li):
                psq = pskv.tile([128, 512], F32, tag="kv", name=f"psq{li}")
                for c in range(CCH):
                    nc.tensor.matmul(psq[:], lhsT=wq_t[:, c * 128:(c + 1) * 128],
                                     rhs=xs[li][c][:, 0:512],
                                     start=(c == 0), stop=(c == CCH - 1))
                # rows 64:127 carry a second q copy (wq = [Wq|Wq]) feeding
                # the row-packed S matmuls' upper-half rhs
                nc.vector.tensor_copy(qt[:, li * 512:(li + 1) * 512], psq[:])

            def emit_proj0():
                # block-0 head: q, kv-h0 and kv-h1 accumulate CONCURRENTLY
                # (3 psum banks), consuming each x chunk-half the moment its
                # DMA lands.  The three PSUM->SBUF copies run on three
                # DIFFERENT engines in parallel; vtrans rides att(0)'s first
                # two windows (O needs vaug only at window 3).
                ps3 = pss.tile([128, 2, 512], F32, tag="s", name="ps3_proj0")
                psq, pkv0 = ps3[:, 0, :], ps3[:, 1, :]
                pkv1 = pskv.tile([128, 512], F32, tag="kv", name="pkv0_1")
                for c in range(CCH):
                    st = dict(start=(c == 0), stop=(c == CCH - 1))
                    nc.tensor.matmul(psq, lhsT=wq_t[:, c * 128:(c + 1) * 128],
                                     rhs=x0h[c][0][:], **st)
                    nc.tensor.matmul(pkv0, lhsT=wkv_t[:, c * 128:(c + 1) * 128],
                                     rhs=x0h[c][0][:], **st)
                for c in range(CCH):
                    nc.tensor.matmul(pkv1, lhsT=wvk_t[:, c * 128:(c + 1) * 128],
                                     rhs=x0h[c][1][:],
                                     start=(c == 0), stop=(c == CCH - 1))
                # gpsimd can't read PSUM: DVE drains both kv halves while
                # the scalar engine (idle until the first exp) drains q
                nc.vector.tensor_copy(kvt[:, 0:512], pkv0)
                nc.vector.tensor_copy(kvt[:, 512:1024], pkv1)
                nc.scalar.copy(qt[:, 0:512], psq)

            def emit_proj_kv(li, half):
                # half 0 -> [k;v] (k in rows 0:64), half 1 -> [v;k] (k in
                # rows 64:128): every S pair is (half0 chunk, half1 chunk),
                # so the pair's two matmuls can row-pack onto disjoint
                # 64-row halves of the PE array.
                pkv = pskv.tile([128, 512], F32, tag="kv", name=f"pkv{li}_{half}")
                w_t = wkv_t if half == 0 else wvk_t
                lo = half * 512
                for c in range(CCH):
                    nc.tensor.matmul(pkv[:], lhsT=w_t[:, c * 128:(c + 1) * 128],
                                     rhs=xs[li][c][:, lo:lo + 512],
                                     start=(c == 0), stop=(c == CCH - 1))
                base = li * 1024 + lo
                nc.vector.tensor_copy(kvt[:, base:base + 512], pkv[:])

            def emit_vtrans(li, half):
                # V rows of the 4 new kv chunks -> vaug (plus ones col, set
                # once above).  All 4 full-chunk transposes land in disjoint
                # slices of one psv tile -> no serialization.  v sits in kv
                # rows 64:128 for half 0 ([k;v]) and 0:64 for half 1
                # ([v;k]) -> transposed, v is cols 64:128 / 0:64.
                pv = psv.tile([128, 512], F16, tag="v", name=f"pv{li}_{half}")
                vofs = 64 if half == 0 else 0
                for k in range(4):
                    sc = li * 8 + half * 4 + k
                    nc.tensor.transpose(
                        pv[:, k * 128:(k + 1) * 128],
                        kvt[:, sc * 128:(sc + 1) * 128], id_t[:])
                    nc.vector.tensor_copy(
                        vaug[:, sc * (H + 1):sc * (H + 1) + H],
                        pv[:, k * 128 + vofs:k * 128 + vofs + 64])

            # ---- attention for q-block li (yields between pair groups so
            # the caller can weave proj(li+1) work into exp-wait bubbles).
            # pending: leftover exp-gated drain/normalize steps of att(li-1),
            # emitted here between li's S-pairs so the in-order PE never
            # stalls on them (their exps are long done by now); this att's
            # own leftovers are appended to pending_out unless li==3.
            def emit_att(li, pending, pending_out):
                po = pso.tile([H + 1, 512], F32, tag="o", name=f"po{li}")
                nfull = 8 * li          # full-width s-chunks (earlier blocks)
                state = {}
                plist = []              # (p, chunk_a, chunk_b, off, w, diag)
                # every pair = (first-half chunk: k in rows 0:64,
                #               second-half chunk: k in rows 64:128)
                for m in range(nfull // 8):
                    for p in range(4):
                        plist.append((4 * m + p, 8 * m + p, 8 * m + p + 4,
                                      0, 512, False))
                for d in range(4):
                    plist.append((nfull // 2 + d, nfull + d, nfull + d + 4,
                                  d * 128, 512 - d * 128, True))
                npairs = len(plist)
                first_chunk = plist[0][1]
                last_chunk = plist[-1][2]

                def emit_spair(i):
                    p, ca, cb, off, w, diag = plist[i]
                    ps_ = pss.tile([128, 2, 512], F32, tag="s",
                                   name=f"ps{li}_{p}")
                    pp = ptile.tile([128, 2, 512], BF16, tag="p",
                                    name=f"pp{li}_{p}")
                    # 2x row-packed: K=64 each, concurrent on disjoint
                    # 64-row groups of the PE array (tile_position derives
                    # from the base partitions: (0,0) and (64,0)).
                    # plane 1 is written COMPACTED at [0:w] so the pair's
                    # exp is one contiguous [off:1024-off] AP (a strided
                    # 3D AP costs the ACT engine ~100ns extra).
                    nc.tensor.matmul(
                        ps_[:, 0, off:512],
                        lhsT=kvt[0:64, ca * 128:(ca + 1) * 128],
                        rhs=qt[0:64, li * 512 + off:(li + 1) * 512],
                        start=True, stop=True)
                    nc.tensor.matmul(
                        ps_[:, 1, 0:w],
                        lhsT=kvt[64:128, cb * 128:(cb + 1) * 128],
                        rhs=qt[64:128, li * 512 + off:(li + 1) * 512],
                        start=True, stop=True)
                    nc.scalar.activation(
                        pp.rearrange("p a b -> p (a b)")[:, off:1024 - off],
                        ps_.rearrange("p a b -> p (a b)")[:, off:1024 - off],
                        EXP, scale=SCALE)
                    if diag:
                        # j=0: own-parity chunk -> causal triangle; j=1:
                        # partner chunk -> all-0 (h=0) / all-1 (h=1) block.
                        # li=3's muls gate the final drain: DVE is idle
                        # there and ~2x faster per op than Pool.  (All-DVE
                        # measured 16us WORSE: Pool's parallelism matters.)
                        eng = nc.vector if li == 3 else nc.gpsimd
                        eng.tensor_mul(pp[:, 0, off:off + 128],
                                       pp[:, 0, off:off + 128],
                                       m01_t[:, 0:128])
                        eng.tensor_mul(pp[:, 1, 0:128],
                                       pp[:, 1, 0:128],
                                       m01_t[:, 128:256])
                    state[i] = pp

                def emit_opair(i):
                    p, ca, cb, off, w, diag = plist[i]
                    pp = state.pop(i)
                    nc.tensor.matmul(
                        po[:, off:512],
                        lhsT=vaug[:, ca * (H + 1):(ca + 1) * (H + 1)],
                        rhs=pp[:, 0, off:512],
                        start=(ca == first_chunk), stop=(ca == last_chunk),
                        skip_group_check=True)
                    nc.tensor.matmul(
                        po[:, off:512],
                        lhsT=vaug[:, cb * (H + 1):(cb + 1) * (H + 1)],
                        rhs=pp[:, 1, 0:w],
                        start=(cb == first_chunk), stop=(cb == last_chunk),
                        skip_group_check=True)

                LA = 3
                on = opost.tile([128, 4 * H], F32, tag="on", name=f"on{li}")
                for i in range(npairs):
                    emit_spair(i)
                    if pending:
                        pending.pop(0)()
                    if i >= LA:
                        emit_opair(i - LA)
                    yield
                steps = [lambda i=i: emit_opair(i)
                         for i in range(npairs - 3, npairs)]
                steps += [lambda k=k: emit_norm_k(li, po, k, on)
                          for k in range(4)]
                if li < 3:
                    pending_out.extend(steps)
                    yield
                    return
                # no fillers here: since the lazy-drain restructure the
                # last exps finish well before these steps, so scratch
                # matmuls would sit serially in the tail.  (Interleaving
                # norm_k behind each drain opair measured no better.)
                for s in steps:
                    s()
                yield

            # normalize + output of one 128-t tile: PSUM->SBUF copy,
            # transpose (alternating psv / the idle pskv bank so tiles
            # k,k+1 don't serialize on one bank's accumulation group),
            # 1/l scale, and a half-size out-DMA every second tile
            def emit_norm_k(li, po, k, on):
                # bf16 through the transpose: po holds unnormalized exp sums
                # up to ~e^51 (no row-max subtraction), so f16 would
                # overflow; bf16 keeps f32 range and the 0.4% step washes
                # out in the Fro-norm (measured rel err stays ~1e-3).
                osb = opost.tile([H + 1, 128], BF16, tag="osb",
                                 name=f"osb{li}_{k}")
                nc.vector.tensor_copy(osb[:], po[:, k * 128:(k + 1) * 128])
                pool = psv if k % 2 == 0 else pskv
                pot = pool.tile([128, H + 1], BF16,
                                tag=("v" if k % 2 == 0 else "kv"),
                                name=f"pot{li}_{k}")
                ptk = pot[:, 0:H + 1]
                nc.tensor.transpose(ptk, osb[:], id32_t[:])
                linv = opost.tile([128, 1], F32, tag="linv",
                                  name=f"linv{li}_{k}")
                nc.vector.reciprocal(linv[:], ptk[:, H:H + 1])
                nc.vector.tensor_scalar_mul(
                    on[:, k * H:(k + 1) * H], ptk[:, 0:H], linv[:])
                if k % 2 == 1:
                    nc.sync.dma_start(
                        out_d.ap()[li * 512 + (k - 1) * 128:
                                   li * 512 + (k + 1) * 128, :].rearrange(
                            "(k2 p) m -> p k2 m", p=128),
                        on[:, (k - 1) * H:(k + 1) * H].rearrange(
                            "p (k2 m) -> p k2 m", k2=2))

            # ---- schedule ----
            # att(li)'s own kv groups ride its first pairs: the q block is
            # the only hard prerequisite of att(li)'s full pairs, so kv(li)
            # can be computed DURING att(li) (its diag pairs come 5th+).
            # This keeps kv groups off att(li-1)'s window (where block-li
            # chunks may not have landed -> in-order PE stall) and gives the
            # exp-bound att windows exp-free PE work to chew on.
            def emit_gkv(li):
                # block 1 must finish kv within att(1)'s windows 1-4 (its
                # diag pairs start at window 5); blocks 2-3 have >=9 full
                # windows, so their kv spreads as 3-MM substeps over 1-6
                # to keep any single window's PE injection small.
                coarse = (li == 1)
                for half in (0, 1):
                    w_t = wkv_t if half == 0 else wvk_t
                    lo = half * 512
                    pkv = pskv.tile([128, 512], F32, tag="kv",
                                    name=f"pkv{li}_{half}")
                    for c in range(3):
                        nc.tensor.matmul(pkv[:],
                                         lhsT=w_t[:, c * 128:(c + 1) * 128],
                                         rhs=xs[li][c][:, lo:lo + 512],
                                         start=(c == 0), stop=False)
                    if not coarse:
                        yield
                    for c in range(3, CCH):
                        nc.tensor.matmul(pkv[:],
                                         lhsT=w_t[:, c * 128:(c + 1) * 128],
                                         rhs=xs[li][c][:, lo:lo + 512],
                                         start=False, stop=(c == CCH - 1))
                    base = li * 1024 + lo
                    nc.vector.tensor_copy(kvt[:, base:base + 512], pkv[:])
                    yield
                    emit_vtrans(li, half)
                    yield

            emit_proj0()

            def gkv0():
                emit_vtrans(0, 0)
                yield
                emit_vtrans(0, 1)
                yield

            pending = []
            gkv = gkv0()
            for li in range(4):
                pending_out = []
                att = emit_att(li, pending, pending_out)
                npairs_li = 4 * li + 4
                i = 0
                for _ in att:
                    i += 1
                    if 1 <= i <= 8:
                        next(gkv, None)        # kv groups of block li
                    # qt(li+1) must be ready before att(li+1) pair 0; for
                    # li=0 wait for the last pair (x(1) DMA lands ~19us --
                    # any earlier and the in-order PE stalls att(0)'s own
                    # remaining S pairs on the x(1) semaphores)
                    if i == (npairs_li if li == 0 else npairs_li - 2) \
                            and li < 3:
                        emit_proj_q(li + 1)
                        gkv = emit_gkv(li + 1)
                for step in pending:   # anything the weave didn't consume
                    step()
                pending = pending_out
            for _ in gkv:
                pass

    nc.compile()
    return nc


def _host_inputs(x, Wq, Wk, Wv):
    """Build the 8 per-core input maps from the full problem inputs."""
    wraw = np.ascontiguousarray(np.concatenate(
        [Wq.T, Wk.T, Wv.T], axis=1).astype(np.float16))  # [C, 192]
    ident = np.eye(128, dtype=np.float16)
    ident32 = np.eye(H + 1, dtype=np.float32).astype(ml_dtypes.bfloat16)
    tri01 = (np.arange(128)[:, None] <= np.arange(128)[None, :])

    in_maps = []
    for ci in range(NCORES):
        b, h = divmod(ci, 2)
        # permuted column order: block li = [4 own q-tiles | 4 partner tiles]
        perm = []
        for li in range(4):
            for k in range(4):
                g = 8 * li + 2 * k + h
                perm.append(np.arange(g * 128, (g + 1) * 128))
            for k in range(4):
                g = 8 * li + 2 * k + (1 - h)
                perm.append(np.arange(g * 128, (g + 1) * 128))
        perm = np.concatenate(perm)
        xt = np.ascontiguousarray(x[b].T.astype(np.float16)[:, perm])  # [C,T]
        # 0/1 P-mask rows: d<4 -> causal triangle (s<=t keeps); d>=4 ->
        # all-zero for h=0 (partner chunk d-4 sits one tile above the
        # diagonal), all-one for h=1 (one tile below)
        m01 = np.empty((128, 2 * 128), np.float32)
        m01[:, 0:128] = tri01
        m01[:, 128:256] = 0.0 if h == 0 else 1.0
        m01 = np.ascontiguousarray(m01).astype(ml_dtypes.bfloat16)
        in_maps.append({
            "xt": xt, "wraw": wraw,
            "m01": m01, "ident": ident, "ident32": ident32,
        })
    return in_maps


def _run(x, Wq, Wk, Wv, trace=False, trace_cores=None):
    if "nc" not in _CACHE:
        _CACHE["nc"] = _build_graph()
    nc = _CACHE["nc"]
    in_maps = _host_inputs(np.asarray(x), np.asarray(Wq),
                           np.asarray(Wk), np.asarray(Wv))
    res = run_bass_kernel_spmd(nc, in_maps, core_ids=list(range(NCORES)),
                               trace=trace, trace_cores=trace_cores)
    out = np.empty((B, T, H), np.float32)
    for ci in range(NCORES):
        b, h = divmod(ci, 2)
        core_out = np.asarray(res.results[ci]["out"])            # [TQ, H]
        for m in range(16):
            g = 2 * m + h
            out[b, g * 128:(g + 1) * 128, :] = \
                core_out[m * 128:(m + 1) * 128, :]
    return out, res


def kernel(x, Wq, Wk, Wv):
    out, _ = _run(x, Wq, Wk, Wv, trace=False)
    return out



# revision 44
# speedup vs baseline: 1.0156x; 1.0156x over previous
"""Distributed single-head causal attention for TRN2 (8 NeuronCores).

Problem: x[B=4, T=4096, C=768], Wq/Wk/Wv[H=64, C] ->
  out[b,t,:] = softmax(causal(q k^T * C^-0.05)) @ v   (single head)

Sharding: core ci = (batch b = ci//2, interleave half h = ci%2). Each core
computes k/v for its whole batch and attention for the 16 q-tiles {2m+h}.

All 8 cores run ONE graph (uniform SPMD); every per-core difference is
carried in per-core DRAM inputs (a per-core COLUMN PERMUTATION of x and the
0/1 P-mask), never in instruction-stream structure or AP offsets.

v3: S matmuls are 2x ROW-PACKED (tile_position).  The S contraction is
only H=64 wide, so two S matmuls (pair elements a, b) run CONCURRENTLY
on disjoint 64-row halves of the PE array: a uses array rows 0:63
(tile_position (0,0)), b uses rows 64:127 ((64,0)).  Layout to feed
them with zero extra copies:
  - wq = [Wq | Wq]: qt rows 64:128 carry a second q copy (was zeros).
  - kv proj weights differ by half: half 0 = [Wk|Wv] (k in rows 0:64),
    half 1 = [Wv|Wk] (k in rows 64:128).  Every pair is (first-half
    chunk, second-half chunk): diag pairs (d, d+4) already were; full
    pairs re-paired (2p,2p+1) -> (8m+p, 8m+p+4).  vtrans picks the v
    rows per half (64:128 for h0, 0:64 for h1).  Masks unchanged.
Effect: an S pair costs ~max(w)/2.4GHz instead of 2w/2.4GHz; measured
PE time drops ~7us and the exp (ACT) stream starts earlier.

v4-v6 (head + stream, from NTFF traces: the kernel is exp/ACT-paced in
steady state -- 40 ACTIVATEs ~= 40us busy at 1 elem/cycle/lane @1.2GHz
-- and the head was DMA-rate-bound):
  - ONE compact [C,192]=[Wq|Wk|Wv] weight DMA (294KB, not 3x196KB);
    the [Wq|Wq]/[Wk|Wv]/[Wv|Wk] matmul layouts are assembled on-chip
    by 6 strided DVE copies.
  - li=0's x arrives as twelve 512-col HALVES spread over THREE DMA
    queues (sync/gpsimd/scalar -- vector can't initiate DMAs; two
    queues only reach ~240GB/s in the ramp window, three ~330).  x(1)
    also rides all three queues so qproj(1) isn't DMA-gated.
  - block-0 head: q, kv-h0, kv-h1 accumulate CONCURRENTLY in 3 psum
    banks, consuming each chunk-half the moment it lands; the three
    PSUM->SBUF drains run on DVE/DVE/scalar in parallel; vtrans rides
    att(0)'s first two windows.  First real exp ~17us (was 24.7).
  - diag S plane 1 is written COMPACTED at [0:w] so every pair's exp
    is one contiguous AP (strided 3D costs ACT ~100ns/instr).
  - norm path in bf16 (po holds unnormalized exp sums up to ~e^51, so
    f16 would overflow): osb/pot/id32 bf16, ~2us less PE.
  - kv of blocks 2-3 weaves as 3-MM substeps over windows 1-6 (block 1
    stays coarse: its kv must finish by window 5).
Scheduling notes from failed experiments (v7-v10, all REGRESSED --
this emission order is load-bearing): deferring block 0's O-pairs,
double-popping pending drains in diag windows, splitting qproj into
2-MM slices, and moving vtrans after both kv copies each made things
1.7-4.5us WORSE; ptile bufs=8 also regressed vs 6.  PSUM is the hard
constraint (8 banks: pss 2x2 + pskv + psv + pso 2) that blocks wider
exp batching; DVE/gpsimd cannot do exp, so ACT ~40us busy is a floor.
Measured v6: 75815ns (v2 baseline 81881ns), rel err 3.26e-3 (bf16
norm; gate 2e-2).  Remaining: ~10us of exp-stream gaps at block
transitions (PE-queue congestion), ~2.8us drain tail, ~4.9us barrier
postamble, ~8us preamble-to-first-DMA.

v2 design (from the v1 trace: PE idle at start, HAM half-clock windows,
exp-paced attention, DMA issue overhead, serialized tail):
  - x is streamed ONCE as 24 [128,1024] f16 chunks (no separate xq stream).
    Host permutes columns per-core so block li = [my 4 q-tiles | partner 4
    tiles]; the q projection reads the fixed [:, 0:512] slice of the same
    chunks the kv projection consumes.  DMA drops 9.2MB -> 6.6MB and all
    chunk DMAs are emitted up front on both queues.
  - exact-causal trim: within the diagonal block, chunk d (0..7) only
    multiplies q-tiles >= tl_min(d); S matmul, exp and O matmul all shrink
    together.  Diagonal chunks pair (d, d+4) -> equal widths -> one strided
    exp per pair ([128,2,w] AP).
  - causal masking = one 128-wide 0/1 bf16 multiply on P per diagonal chunk
    (gpsimd/Pool engine), replacing 256-wide f32 PSUM adds on DVE.
  - warmup burst shrunk 16->6 matmuls (exp-LUT preload kept).
  - tail: O^T transposes land in disjoint slices of one PSUM tile (no
    serialize), one batched output DMA per li ([512,64] each).
  - lazy drains: each li's exp-gated tail O-pairs + normalize are emitted
    between the NEXT li's S-pairs, so the in-order PE never stalls on them.
  - each block's kv groups ride its own attention window (pairs 1-4), not
    the previous one's (whose chunks may not have landed).
Precision: f16 q/k/x/W, bf16 P/V, f32 elsewhere.  No row-max subtraction
(masked scores stay in [-53,51]; exp exact in f32).

Measured on trn2 (neuron-profile, whole NEFF): ~82-83us per core (v1
baseline 92.8-93.8us), rel err 2.28e-3 (gate 2e-2).  Fixed framework
overhead inside the measured window: ~6.8us preamble-to-first-DMA +
~7.5us postamble (8-way engine barrier + ~51 semaphore resets).
Aggregate DMA is ~210GB/s shared across all queues (one AXI port), so
the first ~15us are DMA-bound: scratch 'filler' matmuls pad the PE there
to keep the HAM clock governor at full speed (it demotes the core to
half clock after ~2-3us of PE idleness and needs ~4-5us of sustained
activity to promote).
"""

import sys

for _p in ("/opt/trn_rl_repo",):
    if _p not in sys.path:
        sys.path.insert(0, _p)

import ml_dtypes
import numpy as np

import concourse.bass as bass  # noqa: F401  (registers engine classes)
import concourse.tile as tile
from concourse import bacc, mybir
from concourse.bass_utils import run_bass_kernel_spmd

B, T, C, H = 4, 4096, 768, 64
NCORES = 8
SCALE = float(C ** (-0.05))
CCH = C // 128          # 6 contraction chunks
NSC = T // 128          # 32 s-chunks
TQ = T // 2             # 2048 q columns per core
NWARM = 5               # warmup matmuls (PE clock ramp; first x lands ~9.5us)

F32 = mybir.dt.float32
BF16 = mybir.dt.bfloat16
F16 = mybir.dt.float16
EXP = mybir.ActivationFunctionType.Exp

_CACHE: dict = {}

# diagonal-chunk trim: chunk d of a block only hits q-tiles >= TLMIN[d]
TLMIN = [0, 1, 2, 3, 0, 1, 2, 3]


def _install_ntff_hook():
    """Provide antenv.axon_hooks if the image lacks it, so
    run_bass_kernel_spmd(trace=True) can capture NTFF profiles under axon."""
    try:
        from antenv.axon_hooks import get_axon_ntff_profile_hook  # noqa: F401
        return  # already present
    except ImportError:
        pass
    import contextlib
    import ctypes
    import types

    so_path = "/opt/axon/libaxon_pjrt.so"
    mod = types.ModuleType("antenv.axon_hooks")
    _state = {"hook": None}
    mod.set_axon_ntff_profile_hook = lambda h: _state.__setitem__("hook", h)
    mod.get_axon_ntff_profile_hook = lambda: _state["hook"]
    try:
        lib = ctypes.CDLL(so_path)
        if hasattr(lib, "axon_start_nrt_profile"):
            lib.axon_start_nrt_profile.argtypes = [
                ctypes.POINTER(ctypes.c_int64), ctypes.c_size_t]
            lib.axon_start_nrt_profile.restype = ctypes.c_int64
            lib.axon_stop_nrt_profile.argtypes = [ctypes.c_char_p]
            lib.axon_stop_nrt_profile.restype = ctypes.c_int64

            @contextlib.contextmanager
            def _hook(output_dir, device_ids):
                import jax
                jax.devices()
                if device_ids:
                    ids = (ctypes.c_int64 * len(device_ids))(*device_ids)
                    rc = lib.axon_start_nrt_profile(ids, len(device_ids))
                else:
                    rc = lib.axon_start_nrt_profile(None, 0)
                if rc != 0:
                    raise RuntimeError(f"axon_start_nrt_profile rc={rc}")
                try:
                    yield
                finally:
                    n = lib.axon_stop_nrt_profile(str(output_dir).encode())
                    print(f"profile: {n} file(s) written to {output_dir}")

            _state["hook"] = _hook
    except OSError:
        pass
    import antenv
    sys.modules["antenv.axon_hooks"] = mod
    antenv.axon_hooks = mod


_install_ntff_hook()


def _build_graph():
    nc = bacc.Bacc("TRN2", target_bir_lowering=False, debug=False,
                   num_devices=NCORES)

    xt_d = nc.dram_tensor("xt", [C, T], F16, kind="ExternalInput")
    wraw_d = nc.dram_tensor("wraw", [C, 192], F16, kind="ExternalInput")
    m01_d = nc.dram_tensor("m01", [128, 2 * 128], BF16, kind="ExternalInput")
    id_d = nc.dram_tensor("ident", [128, 128], F16, kind="ExternalInput")
    id32_d = nc.dram_tensor("ident32", [H + 1, H + 1], BF16, kind="ExternalInput")
    out_d = nc.dram_tensor("out", [TQ, H], F32, kind="ExternalOutput")

    with tile.TileContext(nc) as tc:
        with (
            tc.tile_pool(name="consts", bufs=1) as consts,
            tc.tile_pool(name="persist", bufs=1) as persist,
            tc.tile_pool(name="ptile", bufs=6) as ptile,
            tc.tile_pool(name="opost", bufs=4) as opost,
            tc.tile_pool(name="pskv", bufs=1, space="PSUM") as pskv,
            tc.tile_pool(name="psv", bufs=1, space="PSUM") as psv,
            tc.tile_pool(name="pss", bufs=2, space="PSUM") as pss,
            tc.tile_pool(name="pso", bufs=2, space="PSUM") as pso,
        ):
            # ---- constants + x stream, in strict consumption order across
            # both DMA queues (aggregate DMA bw is ~210GB/s shared; the
            # startup is DMA-bound so queue order = need order).
            wq_t = consts.tile([128, CCH * 128], F16, tag="wq", name="wq_t")
            wkv_t = consts.tile([128, CCH * 128], F16, tag="wkv", name="wkv_t")
            wvk_t = consts.tile([128, CCH * 128], F16, tag="wvk", name="wvk_t")
            wraw_t = consts.tile([128, CCH * 192], F16, tag="wraw",
                                 name="wraw_t")
            id_t = consts.tile([128, 128], F16, tag="ident", name="id_t")
            id32_t = consts.tile([H + 1, H + 1], BF16, tag="id32",
                                 name="id32_t")
            m01_t = consts.tile([128, 2 * 128], BF16, tag="m01", name="m01_t")

            # head DMA: the first exp needs weights + ALL of x(li=0), so the
            # head is DMA-rate-bound.  Ship ONE compact [C,192]=[Wq|Wk|Wv]
            # weight tensor (294KB, the [Wq|Wq]/[Wk|Wv]/[Wv|Wk] layouts are
            # assembled on-chip by DVE), and spread li=0's twelve 512-col
            # chunk-halves over FOUR DMA queues (sync/vector/scalar/gpsimd)
            # -- two queues only reach ~240GB/s in the ramp-up window.
            nc.sync.dma_start(
                wraw_t[:].rearrange("p (c m) -> p c m", c=CCH),
                wraw_d.ap().rearrange("(c p) m -> p c m", p=128))
            xs = [[None] * CCH for _ in range(4)]
            x0h = [[None] * 2 for _ in range(CCH)]

            def dma_chunk(li, c, q):
                t_ = persist.tile([128, 1024], F16, tag=f"xs{li}_{c}",
                                  name=f"xs{li}_{c}")
                q.dma_start(t_[:], xt_d.ap()[c * 128:(c + 1) * 128,
                                             li * 1024:(li + 1) * 1024])
                xs[li][c] = t_

            def dma_half(c, half, q):
                t_ = persist.tile([128, 512], F16, tag=f"x0h{c}_{half}",
                                  name=f"x0h{c}_{half}")
                q.dma_start(t_[:], xt_d.ap()[c * 128:(c + 1) * 128,
                                             half * 512:(half + 1) * 512])
                x0h[c][half] = t_

            # need-order: h0 halves feed q+kv0, h1 halves feed kv1.  Only
            # sync/gpsimd/scalar can initiate DMAs.  x(1) also rides all
            # THREE queues: att(0)'s exps end ~21.5us and qproj(1) gates
            # att(1)'s first exp, so x(1) must be fully landed by ~19us.
            # The scalar queue's issue instructions all run before its
            # first real work (table load + exp stream).
            dma_half(0, 0, nc.scalar)
            dma_half(1, 0, nc.gpsimd)
            dma_half(2, 0, nc.sync)
            dma_half(3, 0, nc.scalar)
            dma_half(4, 0, nc.gpsimd)
            dma_half(5, 0, nc.sync)
            dma_half(0, 1, nc.scalar)
            dma_half(1, 1, nc.gpsimd)
            dma_half(2, 1, nc.gpsimd)
            dma_half(3, 1, nc.sync)
            dma_half(4, 1, nc.scalar)
            dma_half(5, 1, nc.gpsimd)
            nc.gpsimd.dma_start(id_t[:], id_d.ap()[:, :])
            nc.gpsimd.dma_start(m01_t[:], m01_d.ap()[:, :])
            dma_chunk(1, 2, nc.sync)
            dma_chunk(1, 1, nc.scalar)
            dma_chunk(1, 0, nc.gpsimd)
            dma_chunk(1, 5, nc.sync)
            dma_chunk(1, 4, nc.scalar)
            dma_chunk(1, 3, nc.gpsimd)
            nc.gpsimd.dma_start(id32_t[:], id32_d.ap()[:, :])
            for li in range(2, 4):
                for c in (0, 2, 4):
                    dma_chunk(li, c, nc.sync)
            for li in range(2, 4):
                for c in (1, 3, 5):
                    dma_chunk(li, c, nc.gpsimd)

            # assemble the 3 weight layouts from the compact DMA (DVE,
            # strided over the 6 contraction chunks; q is duplicated)
            wr3 = wraw_t[:].rearrange("p (c m) -> p c m", c=CCH)
            wq3 = wq_t[:].rearrange("p (c m) -> p c m", c=CCH)
            wkv3 = wkv_t[:].rearrange("p (c m) -> p c m", c=CCH)
            wvk3 = wvk_t[:].rearrange("p (c m) -> p c m", c=CCH)
            nc.vector.tensor_copy(wq3[:, :, 0:64], wr3[:, :, 0:64])
            nc.vector.tensor_copy(wq3[:, :, 64:128], wr3[:, :, 0:64])
            nc.vector.tensor_copy(wkv3[:, :, 0:64], wr3[:, :, 64:128])
            nc.vector.tensor_copy(wkv3[:, :, 64:128], wr3[:, :, 128:192])
            nc.vector.tensor_copy(wvk3[:, :, 0:64], wr3[:, :, 128:192])
            nc.vector.tensor_copy(wvk3[:, :, 64:128], wr3[:, :, 64:128])

            # ---- warmup: preload Exp LUT + wake the PE clock while DMAs
            # stream (writes scratch nothing reads).  fill() emits keep-alive
            # matmuls: the HAM clock governor demotes the core to half speed
            # after ~2us of PE idleness and needs ~5us of sustained activity
            # to promote, so the DMA-bound start is padded with scratch work.
            wsc = persist.tile([128, 512], F16, tag="wsc", name="wsc")
            nc.vector.memset(wsc[:], 0.25)
            wact = persist.tile([128, 64], F32, tag="wact", name="wact")
            nc.vector.memset(wact[:], 0.5)
            nc.scalar.activation(wact[:], wact[:], EXP, scale=SCALE)

            def fill(n):
                for _ in range(n):
                    wps = pss.tile([128, 2, 512], F32, tag="s", name="wps")
                    nc.tensor.matmul(wps[:, 0, :], lhsT=wsc[:, 0:128],
                                     rhs=wsc[:], start=True, stop=True)

            fill(NWARM)

            # ---- persistent intermediates ----
            kvt = persist.tile([128, T], F16, tag="kvt", name="kvt")
            qt = persist.tile([128, TQ], F16, tag="qt", name="qt")
            vaug = persist.tile([128, NSC * (H + 1)], BF16, tag="vaug",
                               name="vaug")
            # ones column of V_aug (accumulates the softmax denominator):
            # single strided memset over all 32 chunks
            nc.vector.memset(vaug.rearrange(
                "p (sc w) -> p sc w", w=H + 1)[:, :, H:H + 1], 1.0)

            # ---- projection phase bodies (per 1024-col block li) ----
            def emit_proj_q(li):
                psq = pskv.tile([128, 512], F32, tag="kv", name=f"psq{li}")
                for c in range(CCH):
                    nc.tensor.matmul(psq[:], lhsT=wq_t[:, c * 128:(c + 1) * 128],
                                     rhs=xs[li][c][:, 0:512],
                                     start=(c == 0), stop=(c == CCH - 1))
                # rows 64:127 carry a second q copy (wq = [Wq|Wq]) feeding
                # the row-packed S matmuls' upper-half rhs
                nc.vector.tensor_copy(qt[:, li * 512:(li + 1) * 512], psq[:])

            def emit_proj0():
                # block-0 head: q, kv-h0 and kv-h1 accumulate CONCURRENTLY
                # (3 psum banks), consuming each x chunk-half the moment its
                # DMA lands.  The three PSUM->SBUF copies run on three
                # DIFFERENT engines in parallel; vtrans rides att(0)'s first
                # two windows (O needs vaug only at window 3).
                ps3 = pss.tile([128, 2, 512], F32, tag="s", name="ps3_proj0")
                psq, pkv0 = ps3[:, 0, :], ps3[:, 1, :]
                pkv1 = pskv.tile([128, 512], F32, tag="kv", name="pkv0_1")
                for c in range(CCH):
                    st = dict(start=(c == 0), stop=(c == CCH - 1))
                    nc.tensor.matmul(psq, lhsT=wq_t[:, c * 128:(c + 1) * 128],
                                     rhs=x0h[c][0][:], **st)
                    nc.tensor.matmul(pkv0, lhsT=wkv_t[:, c * 128:(c + 1) * 128],
                                     rhs=x0h[c][0][:], **st)
                for c in range(CCH):
                    nc.tensor.matmul(pkv1, lhsT=wvk_t[:, c * 128:(c + 1) * 128],
                                     rhs=x0h[c][1][:],
                                     start=(c == 0), stop=(c == CCH - 1))
                # gpsimd can't read PSUM: DVE drains both kv halves while
                # the scalar engine (idle until the first exp) drains q
                nc.vector.tensor_copy(kvt[:, 0:512], pkv0)
                nc.vector.tensor_copy(kvt[:, 512:1024], pkv1)
                nc.scalar.copy(qt[:, 0:512], psq)

            def emit_proj_kv(li, half):
                # half 0 -> [k;v] (k in rows 0:64), half 1 -> [v;k] (k in
                # rows 64:128): every S pair is (half0 chunk, half1 chunk),
                # so the pair's two matmuls can row-pack onto disjoint
                # 64-row halves of the PE array.
                pkv = pskv.tile([128, 512], F32, tag="kv", name=f"pkv{li}_{half}")
                w_t = wkv_t if half == 0 else wvk_t
                lo = half * 512
                for c in range(CCH):
                    nc.tensor.matmul(pkv[:], lhsT=w_t[:, c * 128:(c + 1) * 128],
                                     rhs=xs[li][c][:, lo:lo + 512],
                                     start=(c == 0), stop=(c == CCH - 1))
                base = li * 1024 + lo
                nc.vector.tensor_copy(kvt[:, base:base + 512], pkv[:])

            def emit_vtrans(li, half):
                # V rows of the 4 new kv chunks -> vaug (plus ones col, set
                # once above).  All 4 full-chunk transposes land in disjoint
                # slices of one psv tile -> no serialization.  v sits in kv
                # rows 64:128 for half 0 ([k;v]) and 0:64 for half 1
                # ([v;k]) -> transposed, v is cols 64:128 / 0:64.
                pv = psv.tile([128, 512], F16, tag="v", name=f"pv{li}_{half}")
                vofs = 64 if half == 0 else 0
                for k in range(4):
                    sc = li * 8 + half * 4 + k
                    nc.tensor.transpose(
                        pv[:, k * 128:(k + 1) * 128],
                        kvt[:, sc * 128:(sc + 1) * 128], id_t[:])
                    nc.vector.tensor_copy(
                        vaug[:, sc * (H + 1):sc * (H + 1) + H],
                        pv[:, k * 128 + vofs:k * 128 + vofs + 64])

            # ---- attention for q-block li (yields between pair groups so
            # the caller can weave proj(li+1) work into exp-wait bubbles).
            # pending: leftover exp-gated drain/normalize steps of att(li-1),
            # emitted here between li's S-pairs so the in-order PE never
            # stalls on them (their exps are long done by now); this att's
            # own leftovers are appended to pending_out unless li==3.
            def emit_att(li, pending, pending_out):
                po = pso.tile([H + 1, 512], F32, tag="o", name=f"po{li}")
                nfull = 8 * li          # full-width s-chunks (earlier blocks)
                state = {}
                plist = []              # (p, chunk_a, chunk_b, off, w, diag)
                # every pair = (first-half chunk: k in rows 0:64,
                #               second-half chunk: k in rows 64:128)
                for m in range(nfull // 8):
                    for p in range(4):
                        plist.append((4 * m + p, 8 * m + p, 8 * m + p + 4,
                                      0, 512, False))
                for d in range(4):
                    plist.append((nfull // 2 + d, nfull + d, nfull + d + 4,
                                  d * 128, 512 - d * 128, True))
                npairs = len(plist)
                first_chunk = plist[0][1]
                last_chunk = plist[-1][2]

                def emit_spair(i):
                    p, ca, cb, off, w, diag = plist[i]
                    ps_ = pss.tile([128, 2, 512], F32, tag="s",
                                   name=f"ps{li}_{p}")
                    pp = ptile.tile([128, 2, 512], BF16, tag="p",
                                    name=f"pp{li}_{p}")
                    # 2x row-packed: K=64 each, concurrent on disjoint
                    # 64-row groups of the PE array (tile_position derives
                    # from the base partitions: (0,0) and (64,0)).
                    # plane 1 is written COMPACTED at [0:w] so the pair's
                    # exp is one contiguous [off:1024-off] AP (a strided
                    # 3D AP costs the ACT engine ~100ns extra).
                    nc.tensor.matmul(
                        ps_[:, 0, off:512],
                        lhsT=kvt[0:64, ca * 128:(ca + 1) * 128],
                        rhs=qt[0:64, li * 512 + off:(li + 1) * 512],
                        start=True, stop=True)
                    nc.tensor.matmul(
                        ps_[:, 1, 0:w],
                        lhsT=kvt[64:128, cb * 128:(cb + 1) * 128],
                        rhs=qt[64:128, li * 512 + off:(li + 1) * 512],
                        start=True, stop=True)
                    nc.scalar.activation(
                        pp.rearrange("p a b -> p (a b)")[:, off:1024 - off],
                        ps_.rearrange("p a b -> p (a b)")[:, off:1024 - off],
                        EXP, scale=SCALE)
                    if diag:
                        # j=0: own-parity chunk -> causal triangle; j=1:
                        # partner chunk -> all-0 (h=0) / all-1 (h=1) block.
                        # li=3's muls gate the final drain: DVE is idle
                        # there and ~2x faster per op than Pool.  (All-DVE
                        # measured 16us WORSE: Pool's parallelism matters.)
                        eng = nc.vector if li == 3 else nc.gpsimd
                        eng.tensor_mul(pp[:, 0, off:off + 128],
                                       pp[:, 0, off:off + 128],
                                       m01_t[:, 0:128])
                        eng.tensor_mul(pp[:, 1, 0:128],
                                       pp[:, 1, 0:128],
                                       m01_t[:, 128:256])
                    state[i] = pp

                def emit_opair(i):
                    p, ca, cb, off, w, diag = plist[i]
                    pp = state.pop(i)
                    nc.tensor.matmul(
                        po[:, off:512],
                        lhsT=vaug[:, ca * (H + 1):(ca + 1) * (H + 1)],
                        rhs=pp[:, 0, off:512],
                        start=(ca == first_chunk), stop=(ca == last_chunk),
                        skip_group_check=True)
                    nc.tensor.matmul(
                        po[:, off:512],
                        lhsT=vaug[:, cb * (H + 1):(cb + 1) * (H + 1)],
                        rhs=pp[:, 1, 0:w],
                        start=(cb == first_chunk), stop=(cb == last_chunk),
                        skip_group_check=True)

                LA = 3
                on = opost.tile([128, 4 * H], F32, tag="on", name=f"on{li}")
                for i in range(npairs):
                    emit_spair(i)
                    if pending:
                        pending.pop(0)()
                    if i >= LA:
                        emit_opair(i - LA)
                    yield
                steps = [lambda i=i: emit_opair(i)
                         for i in range(npairs - 3, npairs)]
                steps += [lambda k=k: emit_norm_k(li, po, k, on)
                          for k in range(4)]
                if li < 3:
                    pending_out.extend(steps)
                    yield
                    return
                # no fillers here: since the lazy-drain restructure the
                # last exps finish well before these steps, so scratch
                # matmuls would sit serially in the tail.  (Interleaving
                # norm_k behind each drain opair measured no better.)
                for s in steps:
                    s()
                yield

            # normalize + output of one 128-t tile: PSUM->SBUF copy,
            # transpose (alternating psv / the idle pskv bank so tiles
            # k,k+1 don't serialize on one bank's accumulation group),
            # 1/l scale, and a half-size out-DMA every second tile
            def emit_norm_k(li, po, k, on):
                # bf16 through the transpose: po holds unnormalized exp sums
                # up to ~e^51 (no row-max subtraction), so f16 would
                # overflow; bf16 keeps f32 range and the 0.4% step washes
                # out in the Fro-norm (measured rel err stays ~1e-3).
                osb = opost.tile([H + 1, 128], BF16, tag="osb",
                                 name=f"osb{li}_{k}")
                # block 3's norms run AFTER the last exp: the scalar engine
                # is idle there, so its PSUM drain overlaps DVE's
                # recip/mul chain (DVE was the serial tail bottleneck).
                # Earlier blocks keep DVE -- scalar is the exp pacer then.
                if li == 3:
                    nc.scalar.copy(osb[:], po[:, k * 128:(k + 1) * 128])
                else:
                    nc.vector.tensor_copy(osb[:], po[:, k * 128:(k + 1) * 128])
                pool = psv if k % 2 == 0 else pskv
                pot = pool.tile([128, H + 1], BF16,
                                tag=("v" if k % 2 == 0 else "kv"),
                                name=f"pot{li}_{k}")
                ptk = pot[:, 0:H + 1]
                nc.tensor.transpose(ptk, osb[:], id32_t[:])
                linv = opost.tile([128, 1], F32, tag="linv",
                                  name=f"linv{li}_{k}")
                nc.vector.reciprocal(linv[:], ptk[:, H:H + 1])
                nc.vector.tensor_scalar_mul(
                    on[:, k * H:(k + 1) * H], ptk[:, 0:H], linv[:])
                if k % 2 == 1:
                    nc.sync.dma_start(
                        out_d.ap()[li * 512 + (k - 1) * 128:
                                   li * 512 + (k + 1) * 128, :].rearrange(
                            "(k2 p) m -> p k2 m", p=128),
                        on[:, (k - 1) * H:(k + 1) * H].rearrange(
                            "p (k2 m) -> p k2 m", k2=2))

            # ---- schedule ----
            # att(li)'s own kv groups ride its first pairs: the q block is
            # the only hard prerequisite of att(li)'s full pairs, so kv(li)
            # can be computed DURING att(li) (its diag pairs come 5th+).
            # This keeps kv groups off att(li-1)'s window (where block-li
            # chunks may not have landed -> in-order PE stall) and gives the
            # exp-bound att windows exp-free PE work to chew on.
            def emit_gkv(li):
                # block 1 must finish kv within att(1)'s windows 1-4 (its
                # diag pairs start at window 5); blocks 2-3 have >=9 full
                # windows, so their kv spreads as 3-MM substeps over 1-6
                # to keep any single window's PE injection small.
                coarse = (li == 1)
                for half in (0, 1):
                    w_t = wkv_t if half == 0 else wvk_t
                    lo = half * 512
                    pkv = pskv.tile([128, 512], F32, tag="kv",
                                    name=f"pkv{li}_{half}")
                    for c in range(3):
                        nc.tensor.matmul(pkv[:],
                                         lhsT=w_t[:, c * 128:(c + 1) * 128],
                                         rhs=xs[li][c][:, lo:lo + 512],
                                         start=(c == 0), stop=False)
                    if not coarse:
                        yield
                    for c in range(3, CCH):
                        nc.tensor.matmul(pkv[:],
                                         lhsT=w_t[:, c * 128:(c + 1) * 128],
                                         rhs=xs[li][c][:, lo:lo + 512],
                                         start=False, stop=(c == CCH - 1))
                    base = li * 1024 + lo
                    nc.vector.tensor_copy(kvt[:, base:base + 512], pkv[:])
                    yield
                    emit_vtrans(li, half)
                    yield

            emit_proj0()

            def gkv0():
                emit_vtrans(0, 0)
                yield
                emit_vtrans(0, 1)
                yield

            pending = []
            gkv = gkv0()
            for li in range(4):
                pending_out = []
                att = emit_att(li, pending, pending_out)
                npairs_li = 4 * li + 4
                i = 0
                for _ in att:
                    i += 1
                    if 1 <= i <= 8:
                        next(gkv, None)        # kv groups of block li
                    # qt(li+1) must be ready before att(li+1) pair 0; for
                    # li=0 wait for the last pair (x(1) DMA lands ~19us --
                    # any earlier and the in-order PE stalls att(0)'s own
                    # remaining S pairs on the x(1) semaphores)
                    if i == (npairs_li if li == 0 else npairs_li - 2) \
                            and li < 3:
                        emit_proj_q(li + 1)
                        gkv = emit_gkv(li + 1)
                for step in pending:   # anything the weave didn't consume
                    step()
                pending = pending_out
            for _ in gkv:
                pass

    nc.compile()
    return nc


def _host_inputs(x, Wq, Wk, Wv):
    """Build the 8 per-core input maps from the full problem inputs."""
    wraw = np.ascontiguousarray(np.concatenate(
        [Wq.T, Wk.T, Wv.T], axis=1).astype(np.float16))  # [C, 192]
    ident = np.eye(128, dtype=np.float16)
    ident32 = np.eye(H + 1, dtype=np.float32).astype(ml_dtypes.bfloat16)
    tri01 = (np.arange(128)[:, None] <= np.arange(128)[None, :])

    in_maps = []
    for ci in range(NCORES):
        b, h = divmod(ci, 2)
        # permuted column order: block li = [4 own q-tiles | 4 partner tiles]
        perm = []
        for li in range(4):
            for k in range(4):
                g = 8 * li + 2 * k + h
                perm.append(np.arange(g * 128, (g + 1) * 128))
            for k in range(4):
                g = 8 * li + 2 * k + (1 - h)
                perm.append(np.arange(g * 128, (g + 1) * 128))
        perm = np.concatenate(perm)
        xt = np.ascontiguousarray(x[b].T.astype(np.float16)[:, perm])  # [C,T]
        # 0/1 P-mask rows: d<4 -> causal triangle (s<=t keeps); d>=4 ->
        # all-zero for h=0 (partner chunk d-4 sits one tile above the
        # diagonal), all-one for h=1 (one tile below)
        m01 = np.empty((128, 2 * 128), np.float32)
        m01[:, 0:128] = tri01
        m01[:, 128:256] = 0.0 if h == 0 else 1.0
        m01 = np.ascontiguousarray(m01).astype(ml_dtypes.bfloat16)
        in_maps.append({
            "xt": xt, "wraw": wraw,
            "m01": m01, "ident": ident, "ident32": ident32,
        })
    return in_maps


def _run(x, Wq, Wk, Wv, trace=False, trace_cores=None):
    if "nc" not in _CACHE:
        _CACHE["nc"] = _build_graph()
    nc = _CACHE["nc"]
    in_maps = _host_inputs(np.asarray(x), np.asarray(Wq),
                           np.asarray(Wk), np.asarray(Wv))
    res = run_bass_kernel_spmd(nc, in_maps, core_ids=list(range(NCORES)),
                               trace=trace, trace_cores=trace_cores)
    out = np.empty((B, T, H), np.float32)
    for ci in range(NCORES):
        b, h = divmod(ci, 2)
        core_out = np.asarray(res.results[ci]["out"])            # [TQ, H]
        for m in range(16):
            g = 2 * m + h
            out[b, g * 128:(g + 1) * 128, :] = \
                core_out[m * 128:(m + 1) * 128, :]
    return out, res


def kernel(x, Wq, Wk, Wv):
    out, _ = _run(x, Wq, Wk, Wv, trace=False)
    return out



# revision 47
# speedup vs baseline: 1.0334x; 1.0175x over previous
"""Distributed single-head causal attention for TRN2 (8 NeuronCores).

Problem: x[B=4, T=4096, C=768], Wq/Wk/Wv[H=64, C] ->
  out[b,t,:] = softmax(causal(q k^T * C^-0.05)) @ v   (single head)

Sharding: core ci = (batch b = ci//2, interleave half h = ci%2). Each core
computes k/v for its whole batch and attention for the 16 q-tiles {2m+h}.

All 8 cores run ONE graph (uniform SPMD); every per-core difference is
carried in per-core DRAM inputs (a per-core COLUMN PERMUTATION of x and the
0/1 P-mask), never in instruction-stream structure or AP offsets.

v3: S matmuls are 2x ROW-PACKED (tile_position).  The S contraction is
only H=64 wide, so two S matmuls (pair elements a, b) run CONCURRENTLY
on disjoint 64-row halves of the PE array: a uses array rows 0:63
(tile_position (0,0)), b uses rows 64:127 ((64,0)).  Layout to feed
them with zero extra copies:
  - wq = [Wq | Wq]: qt rows 64:128 carry a second q copy (was zeros).
  - kv proj weights differ by half: half 0 = [Wk|Wv] (k in rows 0:64),
    half 1 = [Wv|Wk] (k in rows 64:128).  Every pair is (first-half
    chunk, second-half chunk): diag pairs (d, d+4) already were; full
    pairs re-paired (2p,2p+1) -> (8m+p, 8m+p+4).  vtrans picks the v
    rows per half (64:128 for h0, 0:64 for h1).  Masks unchanged.
Effect: an S pair costs ~max(w)/2.4GHz instead of 2w/2.4GHz; measured
PE time drops ~7us and the exp (ACT) stream starts earlier.

v4-v6 (head + stream, from NTFF traces: the kernel is exp/ACT-paced in
steady state -- 40 ACTIVATEs ~= 40us busy at 1 elem/cycle/lane @1.2GHz
-- and the head was DMA-rate-bound):
  - ONE compact [C,192]=[Wq|Wk|Wv] weight DMA (294KB, not 3x196KB);
    the [Wq|Wq]/[Wk|Wv]/[Wv|Wk] matmul layouts are assembled on-chip
    by 6 strided DVE copies.
  - li=0's x arrives as twelve 512-col HALVES spread over THREE DMA
    queues (sync/gpsimd/scalar -- vector can't initiate DMAs; two
    queues only reach ~240GB/s in the ramp window, three ~330).  x(1)
    also rides all three queues so qproj(1) isn't DMA-gated.
  - block-0 head: q, kv-h0, kv-h1 accumulate CONCURRENTLY in 3 psum
    banks, consuming each chunk-half the moment it lands; the three
    PSUM->SBUF drains run on DVE/DVE/scalar in parallel; vtrans rides
    att(0)'s first two windows.  First real exp ~17us (was 24.7).
  - diag S plane 1 is written COMPACTED at [0:w] so every pair's exp
    is one contiguous AP (strided 3D costs ACT ~100ns/instr).
  - norm path in bf16 (po holds unnormalized exp sums up to ~e^51, so
    f16 would overflow): osb/pot/id32 bf16, ~2us less PE.
  - kv of blocks 2-3 weaves as 3-MM substeps over windows 1-6 (block 1
    stays coarse: its kv must finish by window 5).
Scheduling notes from failed experiments (v7-v10, all REGRESSED --
this emission order is load-bearing): deferring block 0's O-pairs,
double-popping pending drains in diag windows, splitting qproj into
2-MM slices, and moving vtrans after both kv copies each made things
1.7-4.5us WORSE; ptile bufs=8 also regressed vs 6.  PSUM is the hard
constraint (8 banks: pss 2x2 + pskv + psv + pso 2) that blocks wider
exp batching; DVE/gpsimd cannot do exp, so ACT ~40us busy is a floor.
Measured v6: 75815ns (v2 baseline 81881ns), rel err 3.26e-3 (bf16
norm; gate 2e-2).  Remaining: ~10us of exp-stream gaps at block
transitions (PE-queue congestion), ~2.8us drain tail, ~4.9us barrier
postamble, ~8us preamble-to-first-DMA.

v2 design (from the v1 trace: PE idle at start, HAM half-clock windows,
exp-paced attention, DMA issue overhead, serialized tail):
  - x is streamed ONCE as 24 [128,1024] f16 chunks (no separate xq stream).
    Host permutes columns per-core so block li = [my 4 q-tiles | partner 4
    tiles]; the q projection reads the fixed [:, 0:512] slice of the same
    chunks the kv projection consumes.  DMA drops 9.2MB -> 6.6MB and all
    chunk DMAs are emitted up front on both queues.
  - exact-causal trim: within the diagonal block, chunk d (0..7) only
    multiplies q-tiles >= tl_min(d); S matmul, exp and O matmul all shrink
    together.  Diagonal chunks pair (d, d+4) -> equal widths -> one strided
    exp per pair ([128,2,w] AP).
  - causal masking = one 128-wide 0/1 bf16 multiply on P per diagonal chunk
    (gpsimd/Pool engine), replacing 256-wide f32 PSUM adds on DVE.
  - warmup burst shrunk 16->6 matmuls (exp-LUT preload kept).
  - tail: O^T transposes land in disjoint slices of one PSUM tile (no
    serialize), one batched output DMA per li ([512,64] each).
  - lazy drains: each li's exp-gated tail O-pairs + normalize are emitted
    between the NEXT li's S-pairs, so the in-order PE never stalls on them.
  - each block's kv groups ride its own attention window (pairs 1-4), not
    the previous one's (whose chunks may not have landed).
Precision: f16 q/k/x/W, bf16 P/V, f32 elsewhere.  No row-max subtraction
(masked scores stay in [-53,51]; exp exact in f32).

Measured on trn2 (neuron-profile, whole NEFF): ~82-83us per core (v1
baseline 92.8-93.8us), rel err 2.28e-3 (gate 2e-2).  Fixed framework
overhead inside the measured window: ~6.8us preamble-to-first-DMA +
~7.5us postamble (8-way engine barrier + ~51 semaphore resets).
Aggregate DMA is ~210GB/s shared across all queues (one AXI port), so
the first ~15us are DMA-bound: scratch 'filler' matmuls pad the PE there
to keep the HAM clock governor at full speed (it demotes the core to
half clock after ~2-3us of PE idleness and needs ~4-5us of sustained
activity to promote).
"""

import sys

for _p in ("/opt/trn_rl_repo",):
    if _p not in sys.path:
        sys.path.insert(0, _p)

import ml_dtypes
import numpy as np

import concourse.bass as bass  # noqa: F401  (registers engine classes)
import concourse.tile as tile
from concourse import bacc, mybir
from concourse.bass_utils import run_bass_kernel_spmd

B, T, C, H = 4, 4096, 768, 64
NCORES = 8
SCALE = float(C ** (-0.05))
CCH = C // 128          # 6 contraction chunks
NSC = T // 128          # 32 s-chunks
TQ = T // 2             # 2048 q columns per core
NWARM = 5               # warmup matmuls (PE clock ramp; first x lands ~9.5us)

F32 = mybir.dt.float32
BF16 = mybir.dt.bfloat16
F16 = mybir.dt.float16
EXP = mybir.ActivationFunctionType.Exp

_CACHE: dict = {}

# diagonal-chunk trim: chunk d of a block only hits q-tiles >= TLMIN[d]
TLMIN = [0, 1, 2, 3, 0, 1, 2, 3]


def _install_ntff_hook():
    """Provide antenv.axon_hooks if the image lacks it, so
    run_bass_kernel_spmd(trace=True) can capture NTFF profiles under axon."""
    try:
        from antenv.axon_hooks import get_axon_ntff_profile_hook  # noqa: F401
        return  # already present
    except ImportError:
        pass
    import contextlib
    import ctypes
    import types

    so_path = "/opt/axon/libaxon_pjrt.so"
    mod = types.ModuleType("antenv.axon_hooks")
    _state = {"hook": None}
    mod.set_axon_ntff_profile_hook = lambda h: _state.__setitem__("hook", h)
    mod.get_axon_ntff_profile_hook = lambda: _state["hook"]
    try:
        lib = ctypes.CDLL(so_path)
        if hasattr(lib, "axon_start_nrt_profile"):
            lib.axon_start_nrt_profile.argtypes = [
                ctypes.POINTER(ctypes.c_int64), ctypes.c_size_t]
            lib.axon_start_nrt_profile.restype = ctypes.c_int64
            lib.axon_stop_nrt_profile.argtypes = [ctypes.c_char_p]
            lib.axon_stop_nrt_profile.restype = ctypes.c_int64

            @contextlib.contextmanager
            def _hook(output_dir, device_ids):
                import jax
                jax.devices()
                if device_ids:
                    ids = (ctypes.c_int64 * len(device_ids))(*device_ids)
                    rc = lib.axon_start_nrt_profile(ids, len(device_ids))
                else:
                    rc = lib.axon_start_nrt_profile(None, 0)
                if rc != 0:
                    raise RuntimeError(f"axon_start_nrt_profile rc={rc}")
                try:
                    yield
                finally:
                    n = lib.axon_stop_nrt_profile(str(output_dir).encode())
                    print(f"profile: {n} file(s) written to {output_dir}")

            _state["hook"] = _hook
    except OSError:
        pass
    import antenv
    sys.modules["antenv.axon_hooks"] = mod
    antenv.axon_hooks = mod


_install_ntff_hook()


def _build_graph():
    nc = bacc.Bacc("TRN2", target_bir_lowering=False, debug=False,
                   num_devices=NCORES)

    xt_d = nc.dram_tensor("xt", [C, T], F16, kind="ExternalInput")
    wraw_d = nc.dram_tensor("wraw", [C, 192], F16, kind="ExternalInput")
    m01_d = nc.dram_tensor("m01", [128, 2 * 128], BF16, kind="ExternalInput")
    id_d = nc.dram_tensor("ident", [128, 128], F16, kind="ExternalInput")
    id32_d = nc.dram_tensor("ident32", [H + 1, H + 1], BF16, kind="ExternalInput")
    out_d = nc.dram_tensor("out", [TQ, H], F32, kind="ExternalOutput")

    with tile.TileContext(nc) as tc:
        with (
            tc.tile_pool(name="consts", bufs=1) as consts,
            tc.tile_pool(name="persist", bufs=1) as persist,
            tc.tile_pool(name="ptile", bufs=6) as ptile,
            tc.tile_pool(name="opost", bufs=4) as opost,
            tc.tile_pool(name="pskv", bufs=1, space="PSUM") as pskv,
            tc.tile_pool(name="psv", bufs=1, space="PSUM") as psv,
            tc.tile_pool(name="pss", bufs=2, space="PSUM") as pss,
            tc.tile_pool(name="pso", bufs=2, space="PSUM") as pso,
        ):
            # ---- constants + x stream, in strict consumption order across
            # both DMA queues (aggregate DMA bw is ~210GB/s shared; the
            # startup is DMA-bound so queue order = need order).
            wq_t = consts.tile([128, CCH * 128], F16, tag="wq", name="wq_t")
            wkv_t = consts.tile([128, CCH * 128], F16, tag="wkv", name="wkv_t")
            wvk_t = consts.tile([128, CCH * 128], F16, tag="wvk", name="wvk_t")
            wraw_t = consts.tile([128, CCH * 192], F16, tag="wraw",
                                 name="wraw_t")
            id_t = consts.tile([128, 128], F16, tag="ident", name="id_t")
            id32_t = consts.tile([H + 1, H + 1], BF16, tag="id32",
                                 name="id32_t")
            m01_t = consts.tile([128, 2 * 128], BF16, tag="m01", name="m01_t")

            # head DMA: the first exp needs weights + ALL of x(li=0), so the
            # head is DMA-rate-bound.  Ship ONE compact [C,192]=[Wq|Wk|Wv]
            # weight tensor (294KB, the [Wq|Wq]/[Wk|Wv]/[Wv|Wk] layouts are
            # assembled on-chip by DVE), and spread li=0's twelve 512-col
            # chunk-halves over FOUR DMA queues (sync/vector/scalar/gpsimd)
            # -- two queues only reach ~240GB/s in the ramp-up window.
            nc.sync.dma_start(
                wraw_t[:].rearrange("p (c m) -> p c m", c=CCH),
                wraw_d.ap().rearrange("(c p) m -> p c m", p=128))
            xs = [[None] * CCH for _ in range(4)]
            x0h = [[None] * 2 for _ in range(CCH)]

            def dma_chunk(li, c, q):
                t_ = persist.tile([128, 1024], F16, tag=f"xs{li}_{c}",
                                  name=f"xs{li}_{c}")
                q.dma_start(t_[:], xt_d.ap()[c * 128:(c + 1) * 128,
                                             li * 1024:(li + 1) * 1024])
                xs[li][c] = t_

            def dma_half(c, half, q):
                t_ = persist.tile([128, 512], F16, tag=f"x0h{c}_{half}",
                                  name=f"x0h{c}_{half}")
                q.dma_start(t_[:], xt_d.ap()[c * 128:(c + 1) * 128,
                                             half * 512:(half + 1) * 512])
                x0h[c][half] = t_

            # need-order: h0 halves feed q+kv0, h1 halves feed kv1.  Only
            # sync/gpsimd/scalar can initiate DMAs.  x(1) also rides all
            # THREE queues: att(0)'s exps end ~21.5us and qproj(1) gates
            # att(1)'s first exp, so x(1) must be fully landed by ~19us.
            # The scalar queue's issue instructions all run before its
            # first real work (table load + exp stream).
            dma_half(0, 0, nc.scalar)
            dma_half(1, 0, nc.gpsimd)
            dma_half(2, 0, nc.sync)
            dma_half(3, 0, nc.scalar)
            dma_half(4, 0, nc.gpsimd)
            dma_half(5, 0, nc.sync)
            dma_half(0, 1, nc.scalar)
            dma_half(1, 1, nc.gpsimd)
            dma_half(2, 1, nc.gpsimd)
            dma_half(3, 1, nc.sync)
            dma_half(4, 1, nc.scalar)
            dma_half(5, 1, nc.gpsimd)
            nc.gpsimd.dma_start(id_t[:], id_d.ap()[:, :])
            nc.gpsimd.dma_start(m01_t[:], m01_d.ap()[:, :])
            dma_chunk(1, 2, nc.sync)
            dma_chunk(1, 1, nc.scalar)
            dma_chunk(1, 0, nc.gpsimd)
            dma_chunk(1, 5, nc.sync)
            dma_chunk(1, 4, nc.scalar)
            dma_chunk(1, 3, nc.gpsimd)
            nc.gpsimd.dma_start(id32_t[:], id32_d.ap()[:, :])
            for li in range(2, 4):
                for c in (0, 2, 4):
                    dma_chunk(li, c, nc.sync)
            for li in range(2, 4):
                for c in (1, 3, 5):
                    dma_chunk(li, c, nc.gpsimd)

            # assemble the 3 weight layouts from the compact DMA (DVE,
            # strided over the 6 contraction chunks; q is duplicated)
            wr3 = wraw_t[:].rearrange("p (c m) -> p c m", c=CCH)
            wq3 = wq_t[:].rearrange("p (c m) -> p c m", c=CCH)
            wkv3 = wkv_t[:].rearrange("p (c m) -> p c m", c=CCH)
            wvk3 = wvk_t[:].rearrange("p (c m) -> p c m", c=CCH)
            nc.vector.tensor_copy(wq3[:, :, 0:64], wr3[:, :, 0:64])
            nc.vector.tensor_copy(wq3[:, :, 64:128], wr3[:, :, 0:64])
            nc.vector.tensor_copy(wkv3[:, :, 0:64], wr3[:, :, 64:128])
            nc.vector.tensor_copy(wkv3[:, :, 64:128], wr3[:, :, 128:192])
            nc.vector.tensor_copy(wvk3[:, :, 0:64], wr3[:, :, 128:192])
            nc.vector.tensor_copy(wvk3[:, :, 64:128], wr3[:, :, 64:128])

            # ---- warmup: preload Exp LUT + wake the PE clock while DMAs
            # stream (writes scratch nothing reads).  fill() emits keep-alive
            # matmuls: the HAM clock governor demotes the core to half speed
            # after ~2us of PE idleness and needs ~5us of sustained activity
            # to promote, so the DMA-bound start is padded with scratch work.
            wsc = persist.tile([128, 512], F16, tag="wsc", name="wsc")
            nc.vector.memset(wsc[:], 0.25)
            wact = persist.tile([128, 64], F32, tag="wact", name="wact")
            nc.vector.memset(wact[:], 0.5)
            nc.scalar.activation(wact[:], wact[:], EXP, scale=SCALE)

            def fill(n):
                for _ in range(n):
                    wps = pss.tile([128, 2, 512], F32, tag="s", name="wps")
                    nc.tensor.matmul(wps[:, 0, :], lhsT=wsc[:, 0:128],
                                     rhs=wsc[:], start=True, stop=True)

            def fill_o(n):
                # small keep-warm matmul into the (idle until att) pso bank:
                # proj0 holds pss AND pskv, so fills there would WAR-block
                # the in-order PE behind proj0's own drain copies.
                for _ in range(n):
                    wpo = pso.tile([H + 1, 512], F32, tag="o", name="wpo")
                    nc.tensor.matmul(wpo[:, 0:256], lhsT=wsc[:, 0:H + 1],
                                     rhs=wsc[:, 0:256], start=True, stop=True)

            fill(NWARM)

            # ---- persistent intermediates ----
            kvt = persist.tile([128, T], F16, tag="kvt", name="kvt")
            qt = persist.tile([128, TQ], F16, tag="qt", name="qt")
            vaug = persist.tile([128, NSC * (H + 1)], BF16, tag="vaug",
                               name="vaug")
            # ones column of V_aug (accumulates the softmax denominator):
            # single strided memset over all 32 chunks
            nc.vector.memset(vaug.rearrange(
                "p (sc w) -> p sc w", w=H + 1)[:, :, H:H + 1], 1.0)

            # ---- projection phase bodies (per 1024-col block li) ----
            def emit_proj_q(li):
                psq = pskv.tile([128, 512], F32, tag="kv", name=f"psq{li}")
                for c in range(CCH):
                    nc.tensor.matmul(psq[:], lhsT=wq_t[:, c * 128:(c + 1) * 128],
                                     rhs=xs[li][c][:, 0:512],
                                     start=(c == 0), stop=(c == CCH - 1))
                # rows 64:127 carry a second q copy (wq = [Wq|Wq]) feeding
                # the row-packed S matmuls' upper-half rhs
                nc.vector.tensor_copy(qt[:, li * 512:(li + 1) * 512], psq[:])

            def emit_proj0():
                # block-0 head: q, kv-h0 and kv-h1 accumulate CONCURRENTLY
                # (3 psum banks), consuming each x chunk-half the moment its
                # DMA lands.  The three PSUM->SBUF copies run on three
                # DIFFERENT engines in parallel; vtrans rides att(0)'s first
                # two windows (O needs vaug only at window 3).
                ps3 = pss.tile([128, 2, 512], F32, tag="s", name="ps3_proj0")
                psq, pkv0 = ps3[:, 0, :], ps3[:, 1, :]
                pkv1 = pskv.tile([128, 512], F32, tag="kv", name="pkv0_1")
                # keep-warm fills between the DMA-paced chunk matmuls: the
                # HAM clock governor otherwise never promotes (measured:
                # ALL head MMs ran at 1.2GHz, making the post-last-chunk
                # critical chain to the first exp ~1.3us slower)
                for c in range(CCH):
                    st = dict(start=(c == 0), stop=(c == CCH - 1))
                    nc.tensor.matmul(psq, lhsT=wq_t[:, c * 128:(c + 1) * 128],
                                     rhs=x0h[c][0][:], **st)
                    nc.tensor.matmul(pkv0, lhsT=wkv_t[:, c * 128:(c + 1) * 128],
                                     rhs=x0h[c][0][:], **st)
                    if c % 2 == 1:
                        fill_o(1)
                for c in range(CCH):
                    nc.tensor.matmul(pkv1, lhsT=wvk_t[:, c * 128:(c + 1) * 128],
                                     rhs=x0h[c][1][:],
                                     start=(c == 0), stop=(c == CCH - 1))
                    if c % 2 == 0:
                        fill_o(1)
                # gpsimd can't read PSUM: DVE drains both kv halves while
                # the scalar engine (idle until the first exp) drains q
                nc.vector.tensor_copy(kvt[:, 0:512], pkv0)
                nc.vector.tensor_copy(kvt[:, 512:1024], pkv1)
                nc.scalar.copy(qt[:, 0:512], psq)

            def emit_proj_kv(li, half):
                # half 0 -> [k;v] (k in rows 0:64), half 1 -> [v;k] (k in
                # rows 64:128): every S pair is (half0 chunk, half1 chunk),
                # so the pair's two matmuls can row-pack onto disjoint
                # 64-row halves of the PE array.
                pkv = pskv.tile([128, 512], F32, tag="kv", name=f"pkv{li}_{half}")
                w_t = wkv_t if half == 0 else wvk_t
                lo = half * 512
                for c in range(CCH):
                    nc.tensor.matmul(pkv[:], lhsT=w_t[:, c * 128:(c + 1) * 128],
                                     rhs=xs[li][c][:, lo:lo + 512],
                                     start=(c == 0), stop=(c == CCH - 1))
                base = li * 1024 + lo
                nc.vector.tensor_copy(kvt[:, base:base + 512], pkv[:])

            def emit_vtrans(li, half):
                # V rows of the 4 new kv chunks -> vaug (plus ones col, set
                # once above).  All 4 full-chunk transposes land in disjoint
                # slices of one psv tile -> no serialization.  v sits in kv
                # rows 64:128 for half 0 ([k;v]) and 0:64 for half 1
                # ([v;k]) -> transposed, v is cols 64:128 / 0:64.
                pv = psv.tile([128, 512], F16, tag="v", name=f"pv{li}_{half}")
                vofs = 64 if half == 0 else 0
                for k in range(4):
                    sc = li * 8 + half * 4 + k
                    nc.tensor.transpose(
                        pv[:, k * 128:(k + 1) * 128],
                        kvt[:, sc * 128:(sc + 1) * 128], id_t[:])
                    nc.vector.tensor_copy(
                        vaug[:, sc * (H + 1):sc * (H + 1) + H],
                        pv[:, k * 128 + vofs:k * 128 + vofs + 64])

            # ---- attention for q-block li (yields between pair groups so
            # the caller can weave proj(li+1) work into exp-wait bubbles).
            # pending: leftover exp-gated drain/normalize steps of att(li-1),
            # emitted here between li's S-pairs so the in-order PE never
            # stalls on them (their exps are long done by now); this att's
            # own leftovers are appended to pending_out unless li==3.
            def emit_att(li, pending, pending_out):
                po = pso.tile([H + 1, 512], F32, tag="o", name=f"po{li}")
                nfull = 8 * li          # full-width s-chunks (earlier blocks)
                state = {}
                plist = []              # (p, chunk_a, chunk_b, off, w, diag)
                # every pair = (first-half chunk: k in rows 0:64,
                #               second-half chunk: k in rows 64:128)
                for m in range(nfull // 8):
                    for p in range(4):
                        plist.append((4 * m + p, 8 * m + p, 8 * m + p + 4,
                                      0, 512, False))
                for d in range(4):
                    plist.append((nfull // 2 + d, nfull + d, nfull + d + 4,
                                  d * 128, 512 - d * 128, True))
                npairs = len(plist)
                first_chunk = plist[0][1]
                last_chunk = plist[-1][2]

                def emit_spair(i):
                    p, ca, cb, off, w, diag = plist[i]
                    ps_ = pss.tile([128, 2, 512], F32, tag="s",
                                   name=f"ps{li}_{p}")
                    pp = ptile.tile([128, 2, 512], BF16, tag="p",
                                    name=f"pp{li}_{p}")
                    # 2x row-packed: K=64 each, concurrent on disjoint
                    # 64-row groups of the PE array (tile_position derives
                    # from the base partitions: (0,0) and (64,0)).
                    # plane 1 is written COMPACTED at [0:w] so the pair's
                    # exp is one contiguous [off:1024-off] AP (a strided
                    # 3D AP costs the ACT engine ~100ns extra).
                    nc.tensor.matmul(
                        ps_[:, 0, off:512],
                        lhsT=kvt[0:64, ca * 128:(ca + 1) * 128],
                        rhs=qt[0:64, li * 512 + off:(li + 1) * 512],
                        start=True, stop=True)
                    nc.tensor.matmul(
                        ps_[:, 1, 0:w],
                        lhsT=kvt[64:128, cb * 128:(cb + 1) * 128],
                        rhs=qt[64:128, li * 512 + off:(li + 1) * 512],
                        start=True, stop=True)
                    nc.scalar.activation(
                        pp.rearrange("p a b -> p (a b)")[:, off:1024 - off],
                        ps_.rearrange("p a b -> p (a b)")[:, off:1024 - off],
                        EXP, scale=SCALE)
                    if diag:
                        # j=0: own-parity chunk -> causal triangle; j=1:
                        # partner chunk -> all-0 (h=0) / all-1 (h=1) block.
                        # li=3's muls gate the final drain: DVE is idle
                        # there and ~2x faster per op than Pool.  (All-DVE
                        # measured 16us WORSE: Pool's parallelism matters.)
                        eng = nc.vector if li == 3 else nc.gpsimd
                        eng.tensor_mul(pp[:, 0, off:off + 128],
                                       pp[:, 0, off:off + 128],
                                       m01_t[:, 0:128])
                        eng.tensor_mul(pp[:, 1, 0:128],
                                       pp[:, 1, 0:128],
                                       m01_t[:, 128:256])
                    state[i] = pp

                def emit_opair(i):
                    p, ca, cb, off, w, diag = plist[i]
                    pp = state.pop(i)
                    nc.tensor.matmul(
                        po[:, off:512],
                        lhsT=vaug[:, ca * (H + 1):(ca + 1) * (H + 1)],
                        rhs=pp[:, 0, off:512],
                        start=(ca == first_chunk), stop=(ca == last_chunk),
                        skip_group_check=True)
                    nc.tensor.matmul(
                        po[:, off:512],
                        lhsT=vaug[:, cb * (H + 1):(cb + 1) * (H + 1)],
                        rhs=pp[:, 1, 0:w],
                        start=(cb == first_chunk), stop=(cb == last_chunk),
                        skip_group_check=True)

                LA = 3
                on = opost.tile([128, 4 * H], F32, tag="on", name=f"on{li}")
                for i in range(npairs):
                    emit_spair(i)
                    if pending:
                        pending.pop(0)()
                    if i >= LA:
                        emit_opair(i - LA)
                    yield
                steps = [lambda i=i: emit_opair(i)
                         for i in range(npairs - 3, npairs)]
                steps += [lambda k=k: emit_norm_k(li, po, k, on)
                          for k in range(4)]
                if li < 3:
                    pending_out.extend(steps)
                    yield
                    return
                # no fillers here: since the lazy-drain restructure the
                # last exps finish well before these steps, so scratch
                # matmuls would sit serially in the tail.  (Interleaving
                # norm_k behind each drain opair measured no better.)
                for s in steps:
                    s()
                yield

            # normalize + output of one 128-t tile: PSUM->SBUF copy,
            # transpose (alternating psv / the idle pskv bank so tiles
            # k,k+1 don't serialize on one bank's accumulation group),
            # 1/l scale, and a half-size out-DMA every second tile
            def emit_norm_k(li, po, k, on):
                # bf16 through the transpose: po holds unnormalized exp sums
                # up to ~e^51 (no row-max subtraction), so f16 would
                # overflow; bf16 keeps f32 range and the 0.4% step washes
                # out in the Fro-norm (measured rel err stays ~1e-3).
                osb = opost.tile([H + 1, 128], BF16, tag="osb",
                                 name=f"osb{li}_{k}")
                # block 3's norms run AFTER the last exp: the scalar engine
                # is idle there, so its PSUM drain overlaps DVE's
                # recip/mul chain (DVE was the serial tail bottleneck).
                # Earlier blocks keep DVE -- scalar is the exp pacer then.
                if li == 3:
                    nc.scalar.copy(osb[:], po[:, k * 128:(k + 1) * 128])
                else:
                    nc.vector.tensor_copy(osb[:], po[:, k * 128:(k + 1) * 128])
                pool = psv if k % 2 == 0 else pskv
                pot = pool.tile([128, H + 1], BF16,
                                tag=("v" if k % 2 == 0 else "kv"),
                                name=f"pot{li}_{k}")
                ptk = pot[:, 0:H + 1]
                nc.tensor.transpose(ptk, osb[:], id32_t[:])
                linv = opost.tile([128, 1], F32, tag="linv",
                                  name=f"linv{li}_{k}")
                nc.vector.reciprocal(linv[:], ptk[:, H:H + 1])
                nc.vector.tensor_scalar_mul(
                    on[:, k * H:(k + 1) * H], ptk[:, 0:H], linv[:])
                if k % 2 == 1:
                    nc.sync.dma_start(
                        out_d.ap()[li * 512 + (k - 1) * 128:
                                   li * 512 + (k + 1) * 128, :].rearrange(
                            "(k2 p) m -> p k2 m", p=128),
                        on[:, (k - 1) * H:(k + 1) * H].rearrange(
                            "p (k2 m) -> p k2 m", k2=2))

            # ---- schedule ----
            # att(li)'s own kv groups ride its first pairs: the q block is
            # the only hard prerequisite of att(li)'s full pairs, so kv(li)
            # can be computed DURING att(li) (its diag pairs come 5th+).
            # This keeps kv groups off att(li-1)'s window (where block-li
            # chunks may not have landed -> in-order PE stall) and gives the
            # exp-bound att windows exp-free PE work to chew on.
            def emit_gkv(li):
                # block 1 must finish kv within att(1)'s windows 1-4 (its
                # diag pairs start at window 5); blocks 2-3 have >=9 full
                # windows, so their kv spreads as 3-MM substeps over 1-6
                # to keep any single window's PE injection small.
                coarse = (li == 1)
                for half in (0, 1):
                    w_t = wkv_t if half == 0 else wvk_t
                    lo = half * 512
                    pkv = pskv.tile([128, 512], F32, tag="kv",
                                    name=f"pkv{li}_{half}")
                    for c in range(3):
                        nc.tensor.matmul(pkv[:],
                                         lhsT=w_t[:, c * 128:(c + 1) * 128],
                                         rhs=xs[li][c][:, lo:lo + 512],
                                         start=(c == 0), stop=False)
                    if not coarse:
                        yield
                    for c in range(3, CCH):
                        nc.tensor.matmul(pkv[:],
                                         lhsT=w_t[:, c * 128:(c + 1) * 128],
                                         rhs=xs[li][c][:, lo:lo + 512],
                                         start=False, stop=(c == CCH - 1))
                    base = li * 1024 + lo
                    nc.vector.tensor_copy(kvt[:, base:base + 512], pkv[:])
                    yield
                    emit_vtrans(li, half)
                    yield

            emit_proj0()

            def gkv0():
                emit_vtrans(0, 0)
                yield
                emit_vtrans(0, 1)
                yield

            pending = []
            gkv = gkv0()
            for li in range(4):
                pending_out = []
                att = emit_att(li, pending, pending_out)
                npairs_li = 4 * li + 4
                i = 0
                for _ in att:
                    i += 1
                    if 1 <= i <= 8:
                        next(gkv, None)        # kv groups of block li
                    # qt(li+1) must be ready before att(li+1) pair 0; li=0
                    # triggers at window 3: x(1) lands ~16-18.5us on three
                    # queues, well before this point (~22us), and the
                    # qproj->qt-copy->S chain then overlaps block 0's diag
                    # exp tail instead of following it
                    if i == (3 if li == 0 else npairs_li - 2) and li < 3:
                        emit_proj_q(li + 1)
                        gkv = emit_gkv(li + 1)
                for step in pending:   # anything the weave didn't consume
                    step()
                pending = pending_out
            for _ in gkv:
                pass

    nc.compile()
    return nc


def _host_inputs(x, Wq, Wk, Wv):
    """Build the 8 per-core input maps from the full problem inputs."""
    wraw = np.ascontiguousarray(np.concatenate(
        [Wq.T, Wk.T, Wv.T], axis=1).astype(np.float16))  # [C, 192]
    ident = np.eye(128, dtype=np.float16)
    ident32 = np.eye(H + 1, dtype=np.float32).astype(ml_dtypes.bfloat16)
    tri01 = (np.arange(128)[:, None] <= np.arange(128)[None, :])

    in_maps = []
    for ci in range(NCORES):
        b, h = divmod(ci, 2)
        # permuted column order: block li = [4 own q-tiles | 4 partner tiles]
        perm = []
        for li in range(4):
            for k in range(4):
                g = 8 * li + 2 * k + h
                perm.append(np.arange(g * 128, (g + 1) * 128))
            for k in range(4):
                g = 8 * li + 2 * k + (1 - h)
                perm.append(np.arange(g * 128, (g + 1) * 128))
        perm = np.concatenate(perm)
        xt = np.ascontiguousarray(x[b].T.astype(np.float16)[:, perm])  # [C,T]
        # 0/1 P-mask rows: d<4 -> causal triangle (s<=t keeps); d>=4 ->
        # all-zero for h=0 (partner chunk d-4 sits one tile above the
        # diagonal), all-one for h=1 (one tile below)
        m01 = np.empty((128, 2 * 128), np.float32)
        m01[:, 0:128] = tri01
        m01[:, 128:256] = 0.0 if h == 0 else 1.0
        m01 = np.ascontiguousarray(m01).astype(ml_dtypes.bfloat16)
        in_maps.append({
            "xt": xt, "wraw": wraw,
            "m01": m01, "ident": ident, "ident32": ident32,
        })
    return in_maps


def _run(x, Wq, Wk, Wv, trace=False, trace_cores=None):
    if "nc" not in _CACHE:
        _CACHE["nc"] = _build_graph()
    nc = _CACHE["nc"]
    in_maps = _host_inputs(np.asarray(x), np.asarray(Wq),
                           np.asarray(Wk), np.asarray(Wv))
    res = run_bass_kernel_spmd(nc, in_maps, core_ids=list(range(NCORES)),
                               trace=trace, trace_cores=trace_cores)
    out = np.empty((B, T, H), np.float32)
    for ci in range(NCORES):
        b, h = divmod(ci, 2)
        core_out = np.asarray(res.results[ci]["out"])            # [TQ, H]
        for m in range(16):
            g = 2 * m + h
            out[b, g * 128:(g + 1) * 128, :] = \
                core_out[m * 128:(m + 1) * 128, :]
    return out, res


def kernel(x, Wq, Wk, Wv):
    out, _ = _run(x, Wq, Wk, Wv, trace=False)
    return out

